# revision 1
# baseline (speedup 1.0000x reference)
"""Trainium2 Bass kernel for nn_CNNStateEncoder (dense_cnn).

Network per row (B*S rows, 8 features each):
  conv1 2x2 on [1,2,4] -> 32ch x [1,3]   == h1[96]  = A1[96,8]  @ x[8],  relu(+b1)
  conv2 1x2 on [32,1,3] -> 32ch x [1,2]  == h2[64]  = A2[64,96] @ h1,    relu(+b2)
  linear 64->64                          == out[64] = Wp[64,64] @ h2 + bp

Mapping on each NeuronCore (data parallel over 8 cores, 65536 rows/core,
2048-row tiles; PE HAM stays at 1.2GHz on this part, so minimize matmul
count and maximize row/col-group concurrency):
  - rows live in the matmul free dim (feature-major chain)
  - input: DVE cast f32->bf16, gpsimd x4-replicate into 32-blocks, DVE
    StreamTranspose; row-chunk q's 8 features land at partitions 32q..32q+8
  - conv1: 4 concurrently-packed K=8 matmuls (row groups), one psum bank
    each (concurrent drains must hit distinct banks)
  - relu1: ONE contiguous ACT op over the 4 banks
  - conv2: K=96, N=512 matmuls; the tile's two 1024-row halves go to output
    col groups 0/64 and run concurrently (packed by partition halves)
  - linear: lhsT = activations (M=rows) -> row-major PSUM; 16 chunks issued
    as concurrent (row-group 0-1 x bank0, row-group 2-3 x bank1) pairs
  - out: single DVE bias-add+copy, single 512KB store
"""

import numpy as np
import ml_dtypes

B, S, FEAT, OUT = 64, 8192, 8, 64
NCORES = 8
ROWS_TOTAL = B * S
ROWS_CORE = ROWS_TOTAL // NCORES  # 65536
TILE_ROWS = 2048

BF16 = ml_dtypes.bfloat16

# ---------------------------------------------------------------------------
# numpy-side weight packing
# ---------------------------------------------------------------------------

def pack_weights(W1, b1, W2, b2, Wp, bp):
    W1 = np.asarray(W1, np.float32)
    W2 = np.asarray(W2, np.float32)
    Wp = np.asarray(Wp, np.float32)
    b1 = np.asarray(b1, np.float32)
    b2 = np.asarray(b2, np.float32)
    bp = np.asarray(bp, np.float32)

    # A1 [96, 8]: h1[o*3+j] = sum_{kh,kw} x[kh*4 + j + kw] * W1[o,0,kh,kw]
    A1 = np.zeros((96, 8), np.float32)
    for o in range(32):
        for j in range(3):
            for kh in range(2):
                for kw in range(2):
                    A1[o * 3 + j, kh * 4 + j + kw] += W1[o, 0, kh, kw]
    b1_96 = np.repeat(b1, 3).astype(np.float32)

    # A2 [64, 96]: h2[c*2+w] = sum_{i,kw} h1[i*3 + w + kw] * W2[c,i,0,kw]
    A2 = np.zeros((64, 96), np.float32)
    for c in range(32):
        for w in range(2):
            for i in range(32):
                for kw in range(2):
                    A2[c * 2 + w, i * 3 + w + kw] += W2[c, i, 0, kw]
    b2_64 = np.repeat(b2, 2).astype(np.float32)

    a1t = np.zeros((128, 96), np.float32)
    for q in range(4):
        a1t[32 * q:32 * q + 8, :] = A1.T
    a2t = np.zeros((96, 128), np.float32)
    a2t[:, 0:64] = A2.T
    a2t[:, 64:128] = A2.T
    wpt = np.zeros((128, 64), np.float32)
    wpt[0:64, :] = Wp.T
    wpt[64:128, :] = Wp.T
    b1c = b1_96.reshape(96, 1)
    b2c = np.concatenate([b2_64, b2_64]).reshape(128, 1)
    bpb = np.tile(bp, (128, TILE_ROWS // 128))  # [128, 1024]

    return {
        "a1t": a1t.astype(BF16),
        "a2t": a2t.astype(BF16),
        "wpt": wpt.astype(BF16),
        "b1c": b1c,
        "b2c": b2c,
        "bpb": bpb.astype(np.float32),
    }


# ---------------------------------------------------------------------------
# bass module
# ---------------------------------------------------------------------------

def build_nc(rows=ROWS_CORE):
    import concourse.bass as bass
    import concourse.bacc as bacc
    import concourse.mybir as mybir
    import concourse.tile as tile

    f32 = mybir.dt.float32
    bf16 = mybir.dt.bfloat16
    Relu = mybir.ActivationFunctionType.Relu
    Alu = mybir.AluOpType

    assert rows % TILE_ROWS == 0
    ntiles = rows // TILE_ROWS

    nc = bacc.Bacc(None, target_bir_lowering=False)

    x_d = nc.dram_tensor("x", [rows, FEAT], f32, kind="ExternalInput")
    a1t_d = nc.dram_tensor("a1t", [128, 96], bf16, kind="ExternalInput")
    a2t_d = nc.dram_tensor("a2t", [96, 128], bf16, kind="ExternalInput")
    wpt_d = nc.dram_tensor("wpt", [128, 64], bf16, kind="ExternalInput")
    b1c_d = nc.dram_tensor("b1c", [96, 1], f32, kind="ExternalInput")
    b2c_d = nc.dram_tensor("b2c", [128, 1], f32, kind="ExternalInput")
    bpb_d = nc.dram_tensor("bpb", [128, 1024], f32, kind="ExternalInput")
    out_d = nc.dram_tensor("out", [rows, OUT], f32, kind="ExternalOutput")

    with tile.TileContext(nc) as tc:
        with (
            tc.tile_pool(name="consts", bufs=1) as cpool,
            tc.tile_pool(name="xin", bufs=4) as xpool,
            tc.tile_pool(name="xbf", bufs=4) as xbpool,
            tc.tile_pool(name="xpad", bufs=4) as xppool,
            tc.tile_pool(name="xt", bufs=4) as xtpool,
            tc.tile_pool(name="h1s", bufs=3) as h1pool,
            tc.tile_pool(name="h2s", bufs=3) as h2pool,
            tc.tile_pool(name="osb", bufs=3) as opool,
            tc.tile_pool(name="ps_h1", bufs=1, space="PSUM") as ps_h1,
            tc.tile_pool(name="ps_h2", bufs=1, space="PSUM") as ps_h2,
            tc.tile_pool(name="ps_o", bufs=1, space="PSUM") as ps_o,
        ):
            a1t = cpool.tile([128, 96], bf16)
            a2t = cpool.tile([96, 128], bf16)
            wpt = cpool.tile([128, 64], bf16)
            b1c = cpool.tile([96, 1], f32)
            b2c = cpool.tile([128, 1], f32)
            bpb = cpool.tile([128, 1024], f32)
            nc.sync.dma_start(a1t[:], a1t_d[:])
            nc.sync.dma_start(a2t[:], a2t_d[:])
            nc.sync.dma_start(wpt[:], wpt_d[:])
            nc.sync.dma_start(b1c[:], b1c_d[:])
            nc.sync.dma_start(b2c[:], b2c_d[:])
            nc.sync.dma_start(bpb[:], bpb_d[:])

            for t in range(ntiles):
                n0 = t * TILE_ROWS
                # ---- load + cast + replicate + transpose ----
                x_sb = xpool.tile([128, 128], f32)
                nc.sync.dma_start(
                    x_sb[:],
                    x_d[n0:n0 + TILE_ROWS, :].rearrange("(p r) f -> p (r f)", p=128),
                )
                x_bf = xbpool.tile([128, 128], bf16)
                nc.vector.tensor_copy(x_bf[:], x_sb[:])
                # x_pad[p, 32a+8g+f] = x_bf[p, 8a+f] = x[n0 + 16p + a, f]
                x_pad = xppool.tile([128, 512], bf16)
                rep_ap = (
                    x_bf[:]
                    .rearrange("p (a f) -> p a f", f=8)
                    .unsqueeze(2)
                    .broadcast_to((128, 16, 4, 8))
                )
                nc.gpsimd.tensor_copy(x_pad[:], rep_ap)
                # xt[32q+8g+f, 32a+v] = x[n0 + 512q + 16v + a, f]
                xt = xtpool.tile([128, 512], bf16)
                nc.vector.transpose(xt[:], x_pad[:])

                # ---- conv1: 4 packed K=8 matmuls, one psum bank each ----
                # rhs streams (v outer, a inner) so bank q's col j = row
                # n0 + 512q + j
                h1ps = ps_h1.tile([96, 2048], f32)
                for q in range(4):
                    rhs = xt[32 * q:32 * q + 8, :].rearrange("k (a v) -> k v a", v=32)
                    nc.tensor.matmul(
                        h1ps[:, 512 * q:512 * q + 512],
                        a1t[32 * q:32 * q + 8, :],
                        rhs,
                        tile_position=(32 * q, 0),
                    )
                # ---- relu1 (+b1): ONE contiguous ACT op ----
                h1s = h1pool.tile([96, 2048], bf16)
                nc.scalar.activation(h1s[:], h1ps[:], Relu, bias=b1c[:])

                # ---- conv2: 4 matmuls; the two 1024-row halves of the tile
                # land on col groups 0/64 and run concurrently ----
                h2ps_a = ps_h2.tile([128, 512], f32)
                h2ps_b = ps_h2.tile([128, 512], f32)
                for ps, lo in ((h2ps_a, 0), (h2ps_b, 512)):
                    for h in (0, 1):
                        nc.tensor.matmul(
                            ps[64 * h:64 * h + 64, :],
                            a2t[:, 64 * h:64 * h + 64],
                            h1s[:, 1024 * h + lo:1024 * h + lo + 512],
                            tile_position=(0, 64 * h),
                        )
                # ---- relu2 (+b2): bank A on ACT, bank B on DVE ----
                h2s_a = h2pool.tile([128, 512], bf16)
                h2s_b = h2pool.tile([128, 512], bf16)
                nc.scalar.activation(h2s_a[:], h2ps_a[:], Relu, bias=b2c[:])
                nc.vector.tensor_scalar(
                    h2s_b[:], h2ps_b[:], b2c[:], 0.0, Alu.add, Alu.max
                )

                # ---- linear: 16 chunks of 128 rows; issue (h=0, h=1) chunk
                # pairs adjacently -> concurrent row groups + distinct banks.
                # chunk c covers rows [n0+128c, +128); h = c//8 selects the
                # h2 partition half, bank = c//8 too (cols 64c).
                outps = ps_o.tile([128, 1024], f32)
                for cc in range(8):
                    for h in (0, 1):
                        c = 8 * h + cc
                        X = (c // 4) % 2
                        h2s = h2s_a if X == 0 else h2s_b
                        col = 128 * (c % 4)
                        nc.tensor.matmul(
                            outps[:, 64 * c:64 * c + 64],
                            h2s[64 * h:64 * h + 64, col:col + 128],
                            wpt[64 * h:64 * h + 64, :],
                            start=(cc == 0),
                            stop=(cc == 7),
                            tile_position=(64 * h, 0),
                        )
                # ---- bias + store ----
                out_sb = opool.tile([128, 1024], f32)
                nc.vector.tensor_tensor(out_sb[:], outps[:], bpb[:], Alu.add)
                nc.sync.dma_start(
                    out_d[n0:n0 + TILE_ROWS, :].rearrange("(c p) j -> p c j", p=128),
                    out_sb[:],
                )

    nc.compile()
    return nc


# ---------------------------------------------------------------------------
# entry point
# ---------------------------------------------------------------------------

_CACHE = {}


def _get_nc(rows=ROWS_CORE):
    if rows not in _CACHE:
        _CACHE[rows] = build_nc(rows)
    return _CACHE[rows]


def kernel(x, W1, b1, W2, b2, Wp, bp):
    from concourse.bass_utils import run_bass_kernel_spmd

    x = np.ascontiguousarray(np.asarray(x, np.float32)).reshape(ROWS_TOTAL, FEAT)
    consts = pack_weights(W1, b1, W2, b2, Wp, bp)

    nc = _get_nc()
    in_maps = []
    for c in range(NCORES):
        m = dict(consts)
        m["x"] = x[c * ROWS_CORE:(c + 1) * ROWS_CORE]
        in_maps.append(m)

    res = run_bass_kernel_spmd(nc, in_maps, core_ids=list(range(NCORES)))
    out = np.concatenate([r["out"] for r in res.results], axis=0)
    return out.reshape(B, S, OUT)



# revision 2
# speedup vs baseline: 1.1945x; 1.1945x over previous
"""Trainium2 Bass kernel for nn_CNNStateEncoder (dense_cnn).

Network per row (B*S rows, 8 features each):
  conv1 2x2 on [1,2,4] -> 32ch x [1,3]   == h1[96]  = A1[96,8]  @ x[8],  relu(+b1)
  conv2 1x2 on [32,1,3] -> 32ch x [1,2]  == h2[64]  = A2[64,96] @ h1,    relu(+b2)
  linear 64->64                          == out[64] = Wp[64,64] @ h2 + bp

Data parallel over 8 cores, 65536 rows/core, 2048-row tiles. Feature-major
chain: rows live in the matmul free dim. Per tile:
  - x arrives bf16 (host-cast); gpsimd replicates into 4 32-partition
    blocks; DVE StreamTranspose puts features on partitions
  - conv1: 4 packed K=8 matmuls with CONTIGUOUS rhs streams (the
    resulting fixed row permutation is undone host-side)
  - relu1 (+b1): split Scalar [0:1536] / DVE [1536:2048] for balance
  - conv2: K=96 matmuls, 2 col groups concurrent, one [128,1024] psum
  - relu2 (+b2): single Scalar ACT op over both banks
  - linear: h2 chunks as stationary -> row-major psum [128,1024]
  - bias+cast: single DVE tensor_tensor -> bf16, contiguous 2KB/partition
    store; host unpermutes rows and casts to f32
"""

import numpy as np
import ml_dtypes

B, S, FEAT, OUT = 64, 8192, 8, 64
NCORES = 8
ROWS_TOTAL = B * S
ROWS_CORE = ROWS_TOTAL // NCORES  # 65536
TILE_ROWS = 2048

BF16 = ml_dtypes.bfloat16

# ---------------------------------------------------------------------------
# host-side row permutation
#
# conv1 streams xt columns in natural order, so psum position s holds row
# rho(s) = 512*(s>>9) + 16*(s&31) + ((s&511)>>5)  (within its 2048-row tile).
# The store writes psum (p, chunk r) -> dram slot 16p+r, i.e. dram slot
# d = 16p+r holds true row rho(128r+p).  INV below maps dram order back.
# ---------------------------------------------------------------------------

def _inv_perm():
    d = np.arange(TILE_ROWS)
    p, r = d >> 4, d & 15
    s = 128 * r + p
    rho = 512 * (s >> 9) + 16 * (s & 31) + ((s & 511) >> 5)
    inv = np.empty(TILE_ROWS, np.int64)
    inv[rho] = d
    return inv

INV = _inv_perm()

# ---------------------------------------------------------------------------
# numpy-side weight packing
# ---------------------------------------------------------------------------

def pack_weights(W1, b1, W2, b2, Wp, bp):
    W1 = np.asarray(W1, np.float32)
    W2 = np.asarray(W2, np.float32)
    Wp = np.asarray(Wp, np.float32)
    b1 = np.asarray(b1, np.float32)
    b2 = np.asarray(b2, np.float32)
    bp = np.asarray(bp, np.float32)

    # A1 [96, 8]: h1[o*3+j] = sum_{kh,kw} x[kh*4 + j + kw] * W1[o,0,kh,kw]
    A1 = np.zeros((96, 8), np.float32)
    for o in range(32):
        for j in range(3):
            for kh in range(2):
                for kw in range(2):
                    A1[o * 3 + j, kh * 4 + j + kw] += W1[o, 0, kh, kw]
    b1_96 = np.repeat(b1, 3).astype(np.float32)

    # A2 [64, 96]: h2[c*2+w] = sum_{i,kw} h1[i*3 + w + kw] * W2[c,i,0,kw]
    A2 = np.zeros((64, 96), np.float32)
    for c in range(32):
        for w in range(2):
            for i in range(32):
                for kw in range(2):
                    A2[c * 2 + w, i * 3 + w + kw] += W2[c, i, 0, kw]
    b2_64 = np.repeat(b2, 2).astype(np.float32)

    a1t = np.zeros((128, 96), np.float32)
    for q in range(4):
        a1t[32 * q:32 * q + 8, :] = A1.T
    a2t = np.zeros((96, 128), np.float32)
    a2t[:, 0:64] = A2.T
    a2t[:, 64:128] = A2.T
    wpt = np.zeros((128, 64), np.float32)
    wpt[0:64, :] = Wp.T
    wpt[64:128, :] = Wp.T
    b1c = b1_96.reshape(96, 1)
    b2c = np.concatenate([b2_64, b2_64]).reshape(128, 1)
    bpb = np.tile(bp, (128, TILE_ROWS // 128))  # [128, 1024]

    return {
        "a1t": a1t.astype(BF16),
        "a2t": a2t.astype(BF16),
        "wpt": wpt.astype(BF16),
        "b1c": b1c,
        "b2c": b2c,
        "bpb": bpb.astype(np.float32),
    }


# ---------------------------------------------------------------------------
# bass module
# ---------------------------------------------------------------------------

def build_nc(rows=ROWS_CORE):
    import concourse.bass as bass
    import concourse.bacc as bacc
    import concourse.mybir as mybir
    import concourse.tile as tile

    f32 = mybir.dt.float32
    bf16 = mybir.dt.bfloat16
    Relu = mybir.ActivationFunctionType.Relu
    Alu = mybir.AluOpType

    assert rows % TILE_ROWS == 0
    ntiles = rows // TILE_ROWS

    nc = bacc.Bacc(None, target_bir_lowering=False)

    x_d = nc.dram_tensor("x", [rows, FEAT], bf16, kind="ExternalInput")
    a1t_d = nc.dram_tensor("a1t", [128, 96], bf16, kind="ExternalInput")
    a2t_d = nc.dram_tensor("a2t", [96, 128], bf16, kind="ExternalInput")
    wpt_d = nc.dram_tensor("wpt", [128, 64], bf16, kind="ExternalInput")
    b1c_d = nc.dram_tensor("b1c", [96, 1], f32, kind="ExternalInput")
    b2c_d = nc.dram_tensor("b2c", [128, 1], f32, kind="ExternalInput")
    bpb_d = nc.dram_tensor("bpb", [128, 1024], f32, kind="ExternalInput")
    out_d = nc.dram_tensor("out", [rows, OUT], bf16, kind="ExternalOutput")

    with tile.TileContext(nc) as tc:
        with (
            tc.tile_pool(name="consts", bufs=1) as cpool,
            tc.tile_pool(name="xin", bufs=4) as xpool,
            tc.tile_pool(name="xpad", bufs=4) as xppool,
            tc.tile_pool(name="xt", bufs=4) as xtpool,
            tc.tile_pool(name="h1s", bufs=3) as h1pool,
            tc.tile_pool(name="h2s", bufs=3) as h2pool,
            tc.tile_pool(name="osb", bufs=3) as opool,
            tc.tile_pool(name="ps_h1", bufs=1, space="PSUM") as ps_h1,
            tc.tile_pool(name="ps_h2", bufs=1, space="PSUM") as ps_h2,
            tc.tile_pool(name="ps_o", bufs=1, space="PSUM") as ps_o,
        ):
            a1t = cpool.tile([128, 96], bf16)
            a2t = cpool.tile([96, 128], bf16)
            wpt = cpool.tile([128, 64], bf16)
            b1c = cpool.tile([96, 1], f32)
            b2c = cpool.tile([128, 1], f32)
            bpb = cpool.tile([128, 1024], f32)
            nc.sync.dma_start(a1t[:], a1t_d[:])
            nc.sync.dma_start(a2t[:], a2t_d[:])
            nc.sync.dma_start(wpt[:], wpt_d[:])
            nc.sync.dma_start(b1c[:], b1c_d[:])
            nc.sync.dma_start(b2c[:], b2c_d[:])
            nc.sync.dma_start(bpb[:], bpb_d[:])

            for t in range(ntiles):
                n0 = t * TILE_ROWS
                # ---- load (bf16) + replicate + transpose ----
                x_sb = xpool.tile([128, 128], bf16)
                nc.sync.dma_start(
                    x_sb[:],
                    x_d[n0:n0 + TILE_ROWS, :].rearrange("(p r) f -> p (r f)", p=128),
                )
                # x_pad[p, 32a+8g+f] = x_sb[p, 8a+f] = x[n0 + 16p + a, f]
                x_pad = xppool.tile([128, 512], bf16)
                rep_ap = (
                    x_sb[:]
                    .rearrange("p (a f) -> p a f", f=8)
                    .unsqueeze(2)
                    .broadcast_to((128, 16, 4, 8))
                )
                nc.gpsimd.tensor_copy(x_pad[:], rep_ap)
                # xt[32q+8g+f, 32a+v] = x[n0 + 512q + 16v + a, f]
                xt = xtpool.tile([128, 512], bf16)
                nc.vector.transpose(xt[:], x_pad[:])

                # ---- conv1: 4 packed K=8 matmuls, contiguous rhs ----
                h1ps = ps_h1.tile([96, 2048], f32)
                for q in range(4):
                    nc.tensor.matmul(
                        h1ps[:, 512 * q:512 * q + 512],
                        a1t[32 * q:32 * q + 8, :],
                        xt[32 * q:32 * q + 8, :],
                        tile_position=(32 * q, 0),
                    )
                # ---- relu1 (+b1): Scalar [0:1536], DVE [1536:2048] ----
                h1s = h1pool.tile([96, 2048], bf16)
                nc.scalar.activation(
                    h1s[:, 0:1536], h1ps[:, 0:1536], Relu, bias=b1c[:]
                )
                nc.vector.tensor_scalar(
                    h1s[:, 1536:2048], h1ps[:, 1536:2048], b1c[:], 0.0,
                    Alu.add, Alu.max,
                )

                # ---- conv2: one [128,1024] psum; 2 col groups concurrent ----
                h2ps = ps_h2.tile([128, 1024], f32)
                for lo in (0, 512):
                    for h in (0, 1):
                        nc.tensor.matmul(
                            h2ps[64 * h:64 * h + 64, lo:lo + 512],
                            a2t[:, 64 * h:64 * h + 64],
                            h1s[:, 1024 * h + lo:1024 * h + lo + 512],
                            tile_position=(0, 64 * h),
                        )
                # ---- relu2 (+b2): single Scalar ACT over both banks ----
                h2s = h2pool.tile([128, 1024], bf16)
                nc.scalar.activation(h2s[:], h2ps[:], Relu, bias=b2c[:])

                # ---- linear: 16 chunks of 128 rows (h2 chunk = stationary) ----
                outps = ps_o.tile([128, 1024], f32)
                for cc in range(8):
                    for h in (0, 1):
                        c = 8 * h + cc
                        X = (c // 4) % 2
                        col = 512 * X + 128 * (c % 4)
                        nc.tensor.matmul(
                            outps[:, 64 * c:64 * c + 64],
                            h2s[64 * h:64 * h + 64, col:col + 128],
                            wpt[64 * h:64 * h + 64, :],
                            start=(cc == 0),
                            stop=(cc == 7),
                            tile_position=(64 * h, 0),
                        )
                # ---- bias + cast bf16 + contiguous store ----
                out_sb = opool.tile([128, 1024], bf16)
                nc.vector.tensor_tensor(out_sb[:], outps[:], bpb[:], Alu.add)
                nc.sync.dma_start(
                    out_d[n0:n0 + TILE_ROWS, :].rearrange("(p r) j -> p (r j)", p=128),
                    out_sb[:],
                )

    nc.compile()
    return nc


# ---------------------------------------------------------------------------
# entry point
# ---------------------------------------------------------------------------

_CACHE = {}


def _get_nc(rows=ROWS_CORE):
    if rows not in _CACHE:
        _CACHE[rows] = build_nc(rows)
    return _CACHE[rows]


def make_in_maps(x, W1, b1, W2, b2, Wp, bp):
    x = np.ascontiguousarray(np.asarray(x, np.float32)).reshape(ROWS_TOTAL, FEAT)
    x = x.astype(BF16)
    consts = pack_weights(W1, b1, W2, b2, Wp, bp)
    in_maps = []
    for c in range(NCORES):
        m = dict(consts)
        m["x"] = x[c * ROWS_CORE:(c + 1) * ROWS_CORE]
        in_maps.append(m)
    return in_maps


def postprocess(results):
    out = np.concatenate([np.asarray(r["out"]) for r in results], axis=0)
    out = out.reshape(-1, TILE_ROWS, OUT)[:, INV, :]
    return out.reshape(B, S, OUT).astype(np.float32)


def kernel(x, W1, b1, W2, b2, Wp, bp):
    from concourse.bass_utils import run_bass_kernel_spmd

    nc = _get_nc()
    in_maps = make_in_maps(x, W1, b1, W2, b2, Wp, bp)
    res = run_bass_kernel_spmd(nc, in_maps, core_ids=list(range(NCORES)))
    return postprocess(res.results)


# revision 3
# speedup vs baseline: 1.4258x; 1.1937x over previous
"""Trainium2 Bass kernel for nn_CNNStateEncoder (dense_cnn).

Network per row (B*S rows, 8 features each):
  conv1 2x2 on [1,2,4] -> 32ch x [1,3]   == h1[96]  = A1[96,8]  @ x[8],  relu(+b1)
  conv2 1x2 on [32,1,3] -> 32ch x [1,2]  == h2[64]  = A2[64,96] @ h1,    relu(+b2)
  linear 64->64                          == out[64] = Wp[64,64] @ h2 + bp

Data parallel over 8 cores, 65536 rows/core, 2048-row tiles. Feature-major
chain: rows live in the matmul free dim. The host pre-transposes and
4x-replicates x into the xt layout (xt[32q+8g+f, 32a+v] = x[512q+16v+a, f]
per tile), so the device does no input shuffling at all. Per tile:
  - conv1: 4 packed K=8 matmuls, contiguous rhs streams (row permutation
    undone host-side)
  - relu1 (+b1): single Scalar ACT op [96, 2048]
  - conv2: K=96 matmuls, 2 col groups concurrent, one [128,1024] psum
  - relu2 (+b2): single DVE tensor_scalar [128, 1024]
  - linear: h2 chunks as stationary -> row-major psum [128,1024]
  - bias+cast: single DVE tensor_tensor -> bf16, contiguous 2KB/partition
    store; host unpermutes rows and casts to f32
"""

import numpy as np
import ml_dtypes

B, S, FEAT, OUT = 64, 8192, 8, 64
NCORES = 8
ROWS_TOTAL = B * S
ROWS_CORE = ROWS_TOTAL // NCORES  # 65536
TILE_ROWS = 2048
NTILES = ROWS_CORE // TILE_ROWS  # 32

BF16 = ml_dtypes.bfloat16

# ---------------------------------------------------------------------------
# host-side permutations
#
# conv1 streams xt columns in natural order, so psum position s holds row
# rho(s) = 512*(s>>9) + 16*(s&31) + ((s&511)>>5)  (within its 2048-row tile).
# The store writes psum (p, chunk r) -> dram slot 16p+r, i.e. dram slot
# d = 16p+r holds true row rho(128r+p).  INV below maps dram order back.
# ---------------------------------------------------------------------------

def _inv_perm():
    d = np.arange(TILE_ROWS)
    p, r = d >> 4, d & 15
    s = 128 * r + p
    rho = 512 * (s >> 9) + 16 * (s & 31) + ((s & 511) >> 5)
    inv = np.empty(TILE_ROWS, np.int64)
    inv[rho] = d
    return inv

INV = _inv_perm()


def make_xt(x_core):
    """[ROWS_CORE, 8] bf16 -> [NTILES*128, 512] bf16 in device xt layout."""
    a = x_core.reshape(NTILES, 4, 32, 16, 8)        # (t, q, v, a, f)
    a = a.transpose(0, 1, 4, 3, 2)                  # (t, q, f, a, v)
    a = np.broadcast_to(a[:, :, None], (NTILES, 4, 4, 8, 16, 32))  # (t,q,g,f,a,v)
    return np.ascontiguousarray(a).reshape(NTILES * 128, 512)


# ---------------------------------------------------------------------------
# numpy-side weight packing
# ---------------------------------------------------------------------------

def pack_weights(W1, b1, W2, b2, Wp, bp):
    W1 = np.asarray(W1, np.float32)
    W2 = np.asarray(W2, np.float32)
    Wp = np.asarray(Wp, np.float32)
    b1 = np.asarray(b1, np.float32)
    b2 = np.asarray(b2, np.float32)
    bp = np.asarray(bp, np.float32)

    # A1 [96, 8]: h1[o*3+j] = sum_{kh,kw} x[kh*4 + j + kw] * W1[o,0,kh,kw]
    A1 = np.zeros((96, 8), np.float32)
    for o in range(32):
        for j in range(3):
            for kh in range(2):
                for kw in range(2):
                    A1[o * 3 + j, kh * 4 + j + kw] += W1[o, 0, kh, kw]
    b1_96 = np.repeat(b1, 3).astype(np.float32)

    # A2 [64, 96]: h2[c*2+w] = sum_{i,kw} h1[i*3 + w + kw] * W2[c,i,0,kw]
    A2 = np.zeros((64, 96), np.float32)
    for c in range(32):
        for w in range(2):
            for i in range(32):
                for kw in range(2):
                    A2[c * 2 + w, i * 3 + w + kw] += W2[c, i, 0, kw]
    b2_64 = np.repeat(b2, 2).astype(np.float32)

    a1t = np.zeros((128, 96), np.float32)
    for q in range(4):
        a1t[32 * q:32 * q + 8, :] = A1.T
    a2t = np.zeros((96, 128), np.float32)
    a2t[:, 0:64] = A2.T
    a2t[:, 64:128] = A2.T
    wpt = np.zeros((128, 64), np.float32)
    wpt[0:64, :] = Wp.T
    wpt[64:128, :] = Wp.T
    b1c = b1_96.reshape(96, 1)
    b2c = np.concatenate([b2_64, b2_64]).reshape(128, 1)
    bpb = np.tile(bp, (128, TILE_ROWS // 128))  # [128, 1024]

    return {
        "a1t": a1t.astype(BF16),
        "a2t": a2t.astype(BF16),
        "wpt": wpt.astype(BF16),
        "b1c": b1c,
        "b2c": b2c,
        "bpb": bpb.astype(np.float32),
    }


# ---------------------------------------------------------------------------
# bass module
# ---------------------------------------------------------------------------

def build_nc(rows=ROWS_CORE):
    import concourse.bass as bass
    import concourse.bacc as bacc
    import concourse.mybir as mybir
    import concourse.tile as tile

    f32 = mybir.dt.float32
    bf16 = mybir.dt.bfloat16
    Relu = mybir.ActivationFunctionType.Relu
    Alu = mybir.AluOpType

    assert rows % TILE_ROWS == 0
    ntiles = rows // TILE_ROWS

    nc = bacc.Bacc(None, target_bir_lowering=False)

    xt_d = nc.dram_tensor("xt", [ntiles * 128, 512], bf16, kind="ExternalInput")
    a1t_d = nc.dram_tensor("a1t", [128, 96], bf16, kind="ExternalInput")
    a2t_d = nc.dram_tensor("a2t", [96, 128], bf16, kind="ExternalInput")
    wpt_d = nc.dram_tensor("wpt", [128, 64], bf16, kind="ExternalInput")
    b1c_d = nc.dram_tensor("b1c", [96, 1], f32, kind="ExternalInput")
    b2c_d = nc.dram_tensor("b2c", [128, 1], f32, kind="ExternalInput")
    bpb_d = nc.dram_tensor("bpb", [128, 1024], f32, kind="ExternalInput")
    out_d = nc.dram_tensor("out", [rows, OUT], bf16, kind="ExternalOutput")

    with tile.TileContext(nc) as tc:
        with (
            tc.tile_pool(name="consts", bufs=1) as cpool,
            tc.tile_pool(name="xt", bufs=4) as xtpool,
            tc.tile_pool(name="h1s", bufs=3) as h1pool,
            tc.tile_pool(name="h2s", bufs=3) as h2pool,
            tc.tile_pool(name="osb", bufs=3) as opool,
            tc.tile_pool(name="ps_h1", bufs=1, space="PSUM") as ps_h1,
            tc.tile_pool(name="ps_h2", bufs=1, space="PSUM") as ps_h2,
            tc.tile_pool(name="ps_o", bufs=1, space="PSUM") as ps_o,
        ):
            a1t = cpool.tile([128, 96], bf16)
            a2t = cpool.tile([96, 128], bf16)
            wpt = cpool.tile([128, 64], bf16)
            b1c = cpool.tile([96, 1], f32)
            b2c = cpool.tile([128, 1], f32)
            bpb = cpool.tile([128, 1024], f32)
            nc.sync.dma_start(a1t[:], a1t_d[:])
            nc.sync.dma_start(a2t[:], a2t_d[:])
            nc.sync.dma_start(wpt[:], wpt_d[:])
            nc.sync.dma_start(b1c[:], b1c_d[:])
            nc.sync.dma_start(b2c[:], b2c_d[:])
            nc.sync.dma_start(bpb[:], bpb_d[:])

            for t in range(ntiles):
                n0 = t * TILE_ROWS
                # ---- load pre-transposed xt tile (1KB/partition) ----
                xt = xtpool.tile([128, 512], bf16)
                nc.sync.dma_start(xt[:], xt_d[t * 128:(t + 1) * 128, :])

                # ---- conv1: 4 packed K=8 matmuls, contiguous rhs ----
                h1ps = ps_h1.tile([96, 2048], f32)
                for q in range(4):
                    nc.tensor.matmul(
                        h1ps[:, 512 * q:512 * q + 512],
                        a1t[32 * q:32 * q + 8, :],
                        xt[32 * q:32 * q + 8, :],
                        tile_position=(32 * q, 0),
                    )
                # ---- relu1 (+b1): single Scalar ACT op ----
                h1s = h1pool.tile([96, 2048], bf16)
                nc.scalar.activation(h1s[:], h1ps[:], Relu, bias=b1c[:])

                # ---- conv2: one [128,1024] psum; 2 col groups concurrent ----
                h2ps = ps_h2.tile([128, 1024], f32)
                for lo in (0, 512):
                    for h in (0, 1):
                        nc.tensor.matmul(
                            h2ps[64 * h:64 * h + 64, lo:lo + 512],
                            a2t[:, 64 * h:64 * h + 64],
                            h1s[:, 1024 * h + lo:1024 * h + lo + 512],
                            tile_position=(0, 64 * h),
                        )
                # ---- relu2 (+b2): single DVE tensor_scalar ----
                h2s = h2pool.tile([128, 1024], bf16)
                nc.vector.tensor_scalar(
                    h2s[:], h2ps[:], b2c[:], 0.0, Alu.add, Alu.max
                )

                # ---- linear: 16 chunks of 128 rows (h2 chunk = stationary) ----
                outps = ps_o.tile([128, 1024], f32)
                for cc in range(8):
                    for h in (0, 1):
                        c = 8 * h + cc
                        X = (c // 4) % 2
                        col = 512 * X + 128 * (c % 4)
                        nc.tensor.matmul(
                            outps[:, 64 * c:64 * c + 64],
                            h2s[64 * h:64 * h + 64, col:col + 128],
                            wpt[64 * h:64 * h + 64, :],
                            start=(cc == 0),
                            stop=(cc == 7),
                            tile_position=(64 * h, 0),
                        )
                # ---- bias + cast bf16 + contiguous store ----
                out_sb = opool.tile([128, 1024], bf16)
                nc.vector.tensor_tensor(out_sb[:], outps[:], bpb[:], Alu.add)
                nc.sync.dma_start(
                    out_d[n0:n0 + TILE_ROWS, :].rearrange("(p r) j -> p (r j)", p=128),
                    out_sb[:],
                )

    nc.compile()
    return nc


# ---------------------------------------------------------------------------
# entry point
# ---------------------------------------------------------------------------

_CACHE = {}


def _get_nc(rows=ROWS_CORE):
    if rows not in _CACHE:
        _CACHE[rows] = build_nc(rows)
    return _CACHE[rows]


def make_in_maps(x, W1, b1, W2, b2, Wp, bp):
    x = np.ascontiguousarray(np.asarray(x, np.float32)).reshape(ROWS_TOTAL, FEAT)
    x = x.astype(BF16)
    consts = pack_weights(W1, b1, W2, b2, Wp, bp)
    in_maps = []
    for c in range(NCORES):
        m = dict(consts)
        m["xt"] = make_xt(x[c * ROWS_CORE:(c + 1) * ROWS_CORE])
        in_maps.append(m)
    return in_maps


def postprocess(results):
    out = np.concatenate([np.asarray(r["out"]) for r in results], axis=0)
    out = out.reshape(-1, TILE_ROWS, OUT)[:, INV, :]
    return out.reshape(B, S, OUT).astype(np.float32)


def kernel(x, W1, b1, W2, b2, Wp, bp):
    from concourse.bass_utils import run_bass_kernel_spmd

    nc = _get_nc()
    in_maps = make_in_maps(x, W1, b1, W2, b2, Wp, bp)
    res = run_bass_kernel_spmd(nc, in_maps, core_ids=list(range(NCORES)))
    return postprocess(res.results)


# revision 4
# speedup vs baseline: 1.5380x; 1.0787x over previous
"""Trainium2 Bass kernel for nn_CNNStateEncoder (dense_cnn).

Network per row (B*S rows, 8 features each):
  conv1 2x2 on [1,2,4] -> 32ch x [1,3]   == h1[96]  = A1[96,8]  @ x[8],  relu(+b1)
  conv2 1x2 on [32,1,3] -> 32ch x [1,2]  == h2[64]  = A2[64,96] @ h1,    relu(+b2)
  linear 64->64                          == out[64] = Wp[64,64] @ h2 + bp

Data parallel over 8 cores, 65536 rows/core, 2048-row tiles. Feature-major
chain: rows live in the matmul free dim. The host pre-transposes and
4x-replicates x into the xt layout (xt[32q+8g+f, 32a+v] = x[512q+16v+a, f]
per tile), so the device does no input shuffling at all. Per tile:
  - conv1: 4 packed K=8 matmuls, contiguous rhs streams (row permutation
    undone host-side)
  - relu1 (+b1): single Scalar ACT op [96, 2048]
  - conv2: K=96 matmuls, 2 col groups concurrent, one [128,1024] psum
  - relu2 (+b2): single DVE tensor_scalar [128, 1024]
  - linear: h2 chunks as stationary -> row-major psum [128,1024]
  - bias+cast: single DVE tensor_tensor -> bf16, contiguous 2KB/partition
    store; host unpermutes rows and casts to f32
"""

import numpy as np
import ml_dtypes

B, S, FEAT, OUT = 64, 8192, 8, 64
NCORES = 8
ROWS_TOTAL = B * S
ROWS_CORE = ROWS_TOTAL // NCORES  # 65536
TILE_ROWS = 2048
NTILES = ROWS_CORE // TILE_ROWS  # 32

BF16 = ml_dtypes.bfloat16

# ---------------------------------------------------------------------------
# host-side permutations
#
# conv1 streams xt columns in natural order, so psum position s holds row
# rho(s) = 512*(s>>9) + 16*(s&31) + ((s&511)>>5)  (within its 2048-row tile).
# The store writes psum (p, chunk r) -> dram slot 16p+r, i.e. dram slot
# d = 16p+r holds true row rho(128r+p).  INV below maps dram order back.
# ---------------------------------------------------------------------------

def _inv_perm():
    d = np.arange(TILE_ROWS)
    p, r = d >> 4, d & 15
    s = 128 * r + p
    rho = 512 * (s >> 9) + 16 * (s & 31) + ((s & 511) >> 5)
    inv = np.empty(TILE_ROWS, np.int64)
    inv[rho] = d
    return inv

INV = _inv_perm()


def make_xt(x_core):
    """[ROWS_CORE, 8] bf16 -> [NTILES*128, 512] bf16 in device xt layout."""
    a = x_core.reshape(NTILES, 4, 32, 16, 8)        # (t, q, v, a, f)
    a = a.transpose(0, 1, 4, 3, 2)                  # (t, q, f, a, v)
    a = np.broadcast_to(a[:, :, None], (NTILES, 4, 4, 8, 16, 32))  # (t,q,g,f,a,v)
    return np.ascontiguousarray(a).reshape(NTILES * 128, 512)


# ---------------------------------------------------------------------------
# numpy-side weight packing
# ---------------------------------------------------------------------------

def pack_weights(W1, b1, W2, b2, Wp, bp):
    W1 = np.asarray(W1, np.float32)
    W2 = np.asarray(W2, np.float32)
    Wp = np.asarray(Wp, np.float32)
    b1 = np.asarray(b1, np.float32)
    b2 = np.asarray(b2, np.float32)
    bp = np.asarray(bp, np.float32)

    # A1 [96, 8]: h1[o*3+j] = sum_{kh,kw} x[kh*4 + j + kw] * W1[o,0,kh,kw]
    A1 = np.zeros((96, 8), np.float32)
    for o in range(32):
        for j in range(3):
            for kh in range(2):
                for kw in range(2):
                    A1[o * 3 + j, kh * 4 + j + kw] += W1[o, 0, kh, kw]
    b1_96 = np.repeat(b1, 3).astype(np.float32)

    # A2 [64, 96]: h2[c*2+w] = sum_{i,kw} h1[i*3 + w + kw] * W2[c,i,0,kw]
    A2 = np.zeros((64, 96), np.float32)
    for c in range(32):
        for w in range(2):
            for i in range(32):
                for kw in range(2):
                    A2[c * 2 + w, i * 3 + w + kw] += W2[c, i, 0, kw]
    b2_64 = np.repeat(b2, 2).astype(np.float32)

    a1t = np.zeros((128, 96), np.float32)
    for q in range(4):
        a1t[32 * q:32 * q + 8, :] = A1.T
    a2t = np.zeros((96, 128), np.float32)
    a2t[:, 0:64] = A2.T
    a2t[:, 64:128] = A2.T
    wpt = np.zeros((128, 64), np.float32)
    wpt[0:64, :] = Wp.T
    wpt[64:128, :] = Wp.T
    b1c = b1_96.reshape(96, 1)
    b2c = np.concatenate([b2_64, b2_64]).reshape(128, 1)
    bpb = np.tile(bp, (128, TILE_ROWS // 128))  # [128, 1024]

    return {
        "a1t": a1t.astype(BF16),
        "a2t": a2t.astype(BF16),
        "wpt": wpt.astype(BF16),
        "b1c": b1c,
        "b2c": b2c,
        "bpb": bpb.astype(np.float32),
    }


# ---------------------------------------------------------------------------
# bass module
# ---------------------------------------------------------------------------

def build_nc(rows=ROWS_CORE):
    import concourse.bass as bass
    import concourse.bacc as bacc
    import concourse.mybir as mybir
    import concourse.tile as tile

    f32 = mybir.dt.float32
    bf16 = mybir.dt.bfloat16
    Relu = mybir.ActivationFunctionType.Relu
    Alu = mybir.AluOpType

    assert rows % TILE_ROWS == 0
    ntiles = rows // TILE_ROWS

    nc = bacc.Bacc(None, target_bir_lowering=False)

    xt_d = nc.dram_tensor("xt", [ntiles * 128, 512], bf16, kind="ExternalInput")
    a1t_d = nc.dram_tensor("a1t", [128, 96], bf16, kind="ExternalInput")
    a2t_d = nc.dram_tensor("a2t", [96, 128], bf16, kind="ExternalInput")
    wpt_d = nc.dram_tensor("wpt", [128, 64], bf16, kind="ExternalInput")
    b1c_d = nc.dram_tensor("b1c", [96, 1], f32, kind="ExternalInput")
    b2c_d = nc.dram_tensor("b2c", [128, 1], f32, kind="ExternalInput")
    bpb_d = nc.dram_tensor("bpb", [128, 1024], f32, kind="ExternalInput")
    out_d = nc.dram_tensor("out", [rows, OUT], bf16, kind="ExternalOutput")

    with tile.TileContext(nc) as tc:
        with (
            tc.tile_pool(name="consts", bufs=1) as cpool,
            tc.tile_pool(name="xt", bufs=4) as xtpool,
            tc.tile_pool(name="h1s", bufs=3) as h1pool,
            tc.tile_pool(name="h2s", bufs=3) as h2pool,
            tc.tile_pool(name="osb", bufs=3) as opool,
            tc.tile_pool(name="ps_h1", bufs=1, space="PSUM") as ps_h1,
            tc.tile_pool(name="ps_h2", bufs=1, space="PSUM") as ps_h2,
            tc.tile_pool(name="ps_o", bufs=1, space="PSUM") as ps_o,
        ):
            a1t = cpool.tile([128, 96], bf16)
            a2t = cpool.tile([96, 128], bf16)
            wpt = cpool.tile([128, 64], bf16)
            b1c = cpool.tile([96, 1], f32)
            b2c = cpool.tile([128, 1], f32)
            bpb = cpool.tile([128, 1024], f32)
            nc.sync.dma_start(a1t[:], a1t_d[:])
            nc.sync.dma_start(a2t[:], a2t_d[:])
            nc.sync.dma_start(wpt[:], wpt_d[:])
            nc.sync.dma_start(b1c[:], b1c_d[:])
            nc.sync.dma_start(b2c[:], b2c_d[:])
            nc.sync.dma_start(bpb[:], bpb_d[:])

            def front(t):
                """load + conv1 + relu1 for tile t; returns h1s."""
                xt = xtpool.tile([128, 512], bf16)
                nc.sync.dma_start(xt[:], xt_d[t * 128:(t + 1) * 128, :])
                h1ps = ps_h1.tile([96, 2048], f32)
                for q in range(4):
                    nc.tensor.matmul(
                        h1ps[:, 512 * q:512 * q + 512],
                        a1t[32 * q:32 * q + 8, :],
                        xt[32 * q:32 * q + 8, :],
                        tile_position=(32 * q, 0),
                    )
                # relu1 (+b1): Scalar takes quarters {0,2} (feeding conv2
                # pair lo=0), DVE takes {1,3} (pair lo=512); drains run
                # concurrently and each conv2 pair waits only on its half.
                h1s = h1pool.tile([96, 2048], bf16)
                qa_in = h1ps[:].rearrange("p (h l c) -> p h l c", h=2, l=2)
                qa_out = h1s[:].rearrange("p (h l c) -> p h l c", h=2, l=2)
                nc.scalar.activation(
                    qa_out[:, :, 0], qa_in[:, :, 0], Relu, bias=b1c[:]
                )
                nc.vector.tensor_scalar(
                    qa_out[:, :, 1], qa_in[:, :, 1], b1c[:], 0.0,
                    Alu.add, Alu.max,
                )
                return h1s

            def back(t, h1s):
                """conv2 + relu2 + linear + bias + store for tile t."""
                n0 = t * TILE_ROWS
                h2ps = ps_h2.tile([128, 1024], f32)
                for lo in (0, 512):
                    for h in (0, 1):
                        nc.tensor.matmul(
                            h2ps[64 * h:64 * h + 64, lo:lo + 512],
                            a2t[:, 64 * h:64 * h + 64],
                            h1s[:, 1024 * h + lo:1024 * h + lo + 512],
                            tile_position=(0, 64 * h),
                        )
                # relu2 (+b2): single Scalar ACT op
                h2s = h2pool.tile([128, 1024], bf16)
                nc.scalar.activation(h2s[:], h2ps[:], Relu, bias=b2c[:])

                # linear: 16 chunks of 128 rows (h2 chunk = stationary)
                outps = ps_o.tile([128, 1024], f32)
                for cc in range(8):
                    for h in (0, 1):
                        c = 8 * h + cc
                        X = (c // 4) % 2
                        col = 512 * X + 128 * (c % 4)
                        nc.tensor.matmul(
                            outps[:, 64 * c:64 * c + 64],
                            h2s[64 * h:64 * h + 64, col:col + 128],
                            wpt[64 * h:64 * h + 64, :],
                            start=(cc == 0),
                            stop=(cc == 7),
                            tile_position=(64 * h, 0),
                        )
                # bias + cast bf16 + contiguous store
                out_sb = opool.tile([128, 1024], bf16)
                nc.vector.tensor_tensor(out_sb[:], outps[:], bpb[:], Alu.add)
                nc.sync.dma_start(
                    out_d[n0:n0 + TILE_ROWS, :].rearrange("(p r) j -> p (r j)", p=128),
                    out_sb[:],
                )

            # software-pipelined by one tile: front(t+1) is emitted before
            # back(t) so each engine's FIFO matches dependency order.
            h1s_cur = front(0)
            for t in range(ntiles):
                h1s_next = front(t + 1) if t + 1 < ntiles else None
                back(t, h1s_cur)
                h1s_cur = h1s_next

    nc.compile()
    return nc


# ---------------------------------------------------------------------------
# entry point
# ---------------------------------------------------------------------------

_CACHE = {}


def _get_nc(rows=ROWS_CORE):
    if rows not in _CACHE:
        _CACHE[rows] = build_nc(rows)
    return _CACHE[rows]


def make_in_maps(x, W1, b1, W2, b2, Wp, bp):
    x = np.ascontiguousarray(np.asarray(x, np.float32)).reshape(ROWS_TOTAL, FEAT)
    x = x.astype(BF16)
    consts = pack_weights(W1, b1, W2, b2, Wp, bp)
    in_maps = []
    for c in range(NCORES):
        m = dict(consts)
        m["xt"] = make_xt(x[c * ROWS_CORE:(c + 1) * ROWS_CORE])
        in_maps.append(m)
    return in_maps


def postprocess(results):
    out = np.concatenate([np.asarray(r["out"]) for r in results], axis=0)
    out = out.reshape(-1, TILE_ROWS, OUT)[:, INV, :]
    return out.reshape(B, S, OUT).astype(np.float32)


def kernel(x, W1, b1, W2, b2, Wp, bp):
    from concourse.bass_utils import run_bass_kernel_spmd

    nc = _get_nc()
    in_maps = make_in_maps(x, W1, b1, W2, b2, Wp, bp)
    res = run_bass_kernel_spmd(nc, in_maps, core_ids=list(range(NCORES)))
    return postprocess(res.results)
